# revision 1
# baseline (speedup 1.0000x reference)
"""3-layer GAT (PyG GATConv semantics) on 8 trn2 NeuronCores via Bass/Tile.

Distribution: nodes dst-sharded across the 8 cores (12500 nodes each).
Per layer: local node-phase matmul h_aug = x @ [W | W@As | W@Ad], AllGather
of the per-core h_aug shards into a full DRAM table, then an edge phase over
windows of 128 destination nodes: indirect-DMA gather of h_aug[src] rows,
attention p = exp(leakyrelu(alpha_s[src] + alpha_d[dst])) (segment softmax
without max-subtraction -- exact, logits are O(10)), and the segment scatter
as a PE matmul out += Q^T @ [p*h | p] with Q a one-hot (edge x dst) matrix
built on the vector engine, accumulated in PSUM per window.
"""
import sys
if '/opt/trn_rl_repo' not in sys.path:
    sys.path.insert(0, '/opt/trn_rl_repo')
import numpy as np
from concourse import bass, mybir, tile, bacc

F32 = mybir.dt.float32
I32 = mybir.dt.int32
P = 128
NCORES = 8
N_NODES = 100000
LAST_EXEC_NS = None


# ------------------------------------------------------------- profiling shim

def _install_ntff_hook():
    """Best-effort: register the axon NTFF profile hook if the image's antenv
    lacks it. Returns True if trace=True is usable."""
    try:
        from antenv.axon_hooks import get_axon_ntff_profile_hook  # noqa: F401
        return True
    except ImportError:
        pass
    try:
        import types, contextlib, ctypes, os, json, uuid
        path = "/root/.axon_site/trn_agent_boot/trn_boot.py"
        so = "/opt/axon/libaxon_pjrt.so"
        if not (os.path.exists(path) and os.path.exists(so)):
            return False
        srclines = open(path).read().splitlines()
        start = next(i for i, l in enumerate(srclines)
                     if l.startswith("def _ntff_profile_via_ctypes"))
        end = start + 1
        while end < len(srclines) and (srclines[end].startswith((" ", "\t"))
                                       or not srclines[end].strip()):
            end += 1
        ns = dict(contextlib=contextlib, ctypes=ctypes, sys=sys, os=os,
                  json=json, uuid=uuid)
        exec("\n".join(srclines[start:end]), ns)
        hook = ns["_ntff_profile_via_ctypes"](so)
        mod = types.ModuleType("antenv.axon_hooks")
        mod.get_axon_ntff_profile_hook = lambda: hook
        mod.set_axon_ntff_profile_hook = lambda h: None
        sys.modules["antenv.axon_hooks"] = mod
        return hook is not None
    except Exception:
        return False


# ---------------------------------------------------------------- host side

def preprocess(src, dst, N, ncores):
    nloc = N // ncores
    assert nloc * ncores == N
    nwin = (nloc + P - 1) // P
    npad = nwin * P
    npad_s = npad + 16
    dummy_row = npad
    trows = ncores * npad_s

    src = src.astype(np.int64)
    dst = dst.astype(np.int64)
    core = dst // nloc
    dloc = dst - core * nloc
    win = dloc // P
    rel = (dloc - win * P).astype(np.float32)
    grow = (npad_s * (src // nloc) + (src % nloc)).astype(np.int64)
    edrow = (npad_s * core + dloc).astype(np.int64)

    key = core * nwin + win
    counts = np.bincount(key, minlength=ncores * nwin).reshape(ncores, nwin)
    T = np.maximum(1, (counts.max(axis=0) + P - 1) // P).astype(np.int64)
    slots = (P * T).astype(np.int64)
    win_off = np.concatenate([[0], np.cumsum(slots)]).astype(np.int64)
    slot_tot = int(win_off[-1])

    order = np.argsort(key, kind='stable')
    rank_in_group = np.empty(len(order), np.int64)
    sorted_key = key[order]
    grp_start = np.concatenate([[0], np.flatnonzero(np.diff(sorted_key)) + 1])
    grp_of_sorted = np.repeat(np.arange(len(grp_start)),
                              np.diff(np.concatenate([grp_start, [len(order)]])))
    rank_in_group[order] = np.arange(len(order)) - grp_start[grp_of_sorted]

    DUMMY_G = dummy_row          # dummy row in core 0's shard region
    g_idx = np.full((ncores, slot_tot), DUMMY_G, np.int32)
    ed_idx = np.full((ncores, slot_tot), DUMMY_G, np.int32)
    rel_f = np.full((ncores, slot_tot), -1.0, np.float32)

    s = rank_in_group
    pcol = s % P
    j = s // P
    pos = win_off[win] + pcol * T[win] + j
    g_idx[core, pos] = grow
    ed_idx[core, pos] = edrow
    rel_f[core, pos] = rel

    meta = dict(N=N, ncores=ncores, nloc=nloc, nwin=int(nwin), npad=int(npad),
                npad_s=int(npad_s), trows=int(trows), dummy=int(dummy_row),
                T=[int(t) for t in T], win_off=[int(o) for o in win_off],
                slot_tot=slot_tot)
    return meta, g_idx, ed_idx, rel_f


def make_weights(inp):
    def aug(W, a_s, a_d):
        H, C = a_s.shape
        As = np.zeros((H * C, H), np.float32)
        Ad = np.zeros((H * C, H), np.float32)
        for h in range(H):
            As[h * C:(h + 1) * C, h] = a_s[h]
            Ad[h * C:(h + 1) * C, h] = a_d[h]
        return np.concatenate([W, W @ As, W @ Ad], axis=1).astype(np.float32)
    return (aug(np.asarray(inp['W0'], np.float32), np.asarray(inp['as0']),
                np.asarray(inp['ad0'])),
            aug(np.asarray(inp['W1'], np.float32), np.asarray(inp['as1']),
                np.asarray(inp['ad1'])),
            aug(np.asarray(inp['W2'], np.float32), np.asarray(inp['as2']),
                np.asarray(inp['ad2'])))


def make_const_inputs(inp):
    Waug0, Waug1, Waug2 = make_weights(inp)
    FA, FA2 = 136, 34
    dummy = np.zeros((16, FA), np.float32)
    dummy[:, 128:132] = -1e30
    dummy2 = np.zeros((16, FA2), np.float32)
    dummy2[:, 32:33] = -1e30
    tl = lambda a: np.tile(np.asarray(a, np.float32).reshape(1, -1), (P, 1))
    return dict(
        Waug0=Waug0, Waug1=Waug1, Waug2=Waug2,
        b0=tl(inp['b0']), b1=tl(inp['b1']), b2=tl(inp['b2']),
        linw=np.asarray(inp['lin_w'], np.float32),
        linb=tl(inp['lin_b']),
        iota=np.tile(np.arange(P, dtype=np.float32).reshape(1, P), (P, 1)),
        dummyrow=dummy, dummyrow2=dummy2,
        ident=np.eye(P, dtype=np.float32),
    )


# ---------------------------------------------------------------- device side

def ap_nd(t_ap, off, dims):
    """AP over the same tensor: keep partition dim, explicit free dims."""
    ap = [list(t_ap.ap[0])] + [[int(s), int(n)] for (s, n) in dims]
    return bass.AP(t_ap.tensor, t_ap.offset + off, ap)


def build_program(meta, ncores=None):
    ncores = ncores or meta['ncores']
    nwin, npad, npad_s, trows = (meta['nwin'], meta['npad'], meta['npad_s'],
                                 meta['trows'])
    T, win_off, slot_tot = meta['T'], meta['win_off'], meta['slot_tot']
    FH, H, C = 128, 4, 32
    FA = FH + 2 * H
    FA2 = C + 2
    NCLS = 40

    nc = bacc.Bacc("TRN2", target_bir_lowering=False, debug=False,
                   num_devices=ncores)
    dp = nc.declare_dram_parameter
    xT = dp("xT", [P, npad], F32, isOutput=False)
    gidx_d = dp("gidx", [slot_tot], I32, isOutput=False)
    edidx_d = dp("edidx", [slot_tot], I32, isOutput=False)
    rel_d = dp("rel", [slot_tot], F32, isOutput=False)
    Waug0_d = dp("Waug0", [P, FA], F32, isOutput=False)
    Waug1_d = dp("Waug1", [P, FA], F32, isOutput=False)
    Waug2_d = dp("Waug2", [P, FA2], F32, isOutput=False)
    b0_d = dp("b0", [P, FH], F32, isOutput=False)
    b1_d = dp("b1", [P, FH], F32, isOutput=False)
    b2_d = dp("b2", [P, C], F32, isOutput=False)
    linw_d = dp("linw", [C, NCLS], F32, isOutput=False)
    linb_d = dp("linb", [P, NCLS], F32, isOutput=False)
    iota_d = dp("iota", [P, P], F32, isOutput=False)
    dummy_d = dp("dummyrow", [16, FA], F32, isOutput=False)
    dummy2_d = dp("dummyrow2", [16, FA2], F32, isOutput=False)
    ident_d = dp("ident", [P, P], F32, isOutput=False)
    out_ext = dp("out", [npad, NCLS], F32, isOutput=True)

    rg = [list(range(ncores))]

    with tile.TileContext(nc) as tc:
        with tc.tile_pool(name="dram", bufs=1, space="DRAM") as dram, \
             tc.tile_pool(name="consts", bufs=1) as cp, \
             tc.tile_pool(name="work", bufs=3) as wp, \
             tc.tile_pool(name="psum", bufs=2, space="PSUM") as pp:

            table0 = dram.tile([trows, FA], F32, addr_space="Shared",
                               name="table0")
            table1 = dram.tile([trows, FA], F32, addr_space="Shared",
                               name="table1")
            table2 = dram.tile([trows, FA2], F32, addr_space="Shared",
                               name="table2")
            shard0 = dram.tile([npad_s, FA], F32, name="shard0")
            shard1 = dram.tile([npad_s, FA], F32, name="shard1")
            shard2 = dram.tile([npad_s, FA2], F32, name="shard2")

            def cload(dram_ap, shape, name):
                t = cp.tile(shape, F32, name=name, tag=name)
                nc.sync.dma_start(out=t[:], in_=dram_ap)
                return t
            Waug0_s = cload(Waug0_d[:], [P, FA], "Waug0_s")
            Waug1_s = cload(Waug1_d[:], [P, FA], "Waug1_s")
            Waug2_s = cload(Waug2_d[:], [P, FA2], "Waug2_s")
            b0_s = cload(b0_d[:], [P, FH], "b0_s")
            b1_s = cload(b1_d[:], [P, FH], "b1_s")
            b2_s = cload(b2_d[:], [P, C], "b2_s")
            linw_s = cload(linw_d[:], [C, NCLS], "linw_s")
            linb_s = cload(linb_d[:], [P, NCLS], "linb_s")
            iota_s = cload(iota_d[:], [P, P], "iota_s")
            dummy_s = cload(dummy_d[:], [16, FA], "dummy_s")
            dummy2_s = cload(dummy2_d[:], [16, FA2], "dummy2_s")
            ident_s = cload(ident_d[:], [P, P], "ident_s")

            # node phase layer 0
            for blk in range(nwin):
                xT_t = wp.tile([P, P], F32, tag="xT_t")
                nc.sync.dma_start(out=xT_t[:], in_=xT[:, blk * P:(blk + 1) * P])
                ps = pp.tile([P, FA], F32, tag="ps_node")
                nc.tensor.matmul(out=ps[:], lhsT=xT_t[:], rhs=Waug0_s[:],
                                 start=True, stop=True)
                hsb = wp.tile([P, FA], F32, tag="hsb")
                nc.scalar.copy(out=hsb[:], in_=ps[:])
                nc.sync.dma_start(out=shard0[blk * P:(blk + 1) * P, :],
                                  in_=hsb[:])
            nc.sync.dma_start(out=shard0[npad:npad + 16, :], in_=dummy_s[:])

            def allgather(shard, table):
                nc.gpsimd.collective_compute(
                    "AllGather", mybir.AluOpType.bypass,
                    replica_groups=rg, ins=[shard.opt()], outs=[table.opt()])

            allgather(shard0, table0)

            def edge_phase(table, FT, nheads, shard_next, FN, Waug_next_s,
                           b_s, final):
                ch = C
                fh = nheads * ch
                cols = fh + nheads
                for w in range(nwin):
                    J = T[w]
                    off = win_off[w]
                    nslots = P * J
                    gi0 = wp.tile([P, J], I32, tag="gi0")
                    nc.sync.dma_start(
                        out=gi0[:], in_=gidx_d[off:off + nslots].rearrange(
                            "(p j) -> p j", p=P))
                    gi = wp.tile([P, J], I32, tag="gi")
                    nc.vector.tensor_copy(out=gi[:], in_=gi0[:])
                    ei0 = wp.tile([P, J], I32, tag="ei0")
                    nc.sync.dma_start(
                        out=ei0[:], in_=edidx_d[off:off + nslots].rearrange(
                            "(p j) -> p j", p=P))
                    ei = wp.tile([P, J], I32, tag="ei")
                    nc.vector.tensor_copy(out=ei[:], in_=ei0[:])
                    rl = wp.tile([P, J], F32, tag="rl")
                    nc.sync.dma_start(
                        out=rl[:], in_=rel_d[off:off + nslots].rearrange(
                            "(p j) -> p j", p=P))
                    pay = wp.tile([P, J * FT], F32, tag="pay")
                    edv = wp.tile([P, J * nheads], F32, tag="edv")
                    for j in range(J):
                        nc.gpsimd.indirect_dma_start(
                            out=pay[:, j * FT:(j + 1) * FT], out_offset=None,
                            in_=table[:],
                            in_offset=bass.IndirectOffsetOnAxis(
                                ap=gi[:, j:j + 1], axis=0),
                            oob_is_err=False)
                        nc.gpsimd.indirect_dma_start(
                            out=edv[:, j * nheads:(j + 1) * nheads],
                            out_offset=None, in_=table[:],
                            in_offset=bass.IndirectOffsetOnAxis(
                                ap=ei[:, j:j + 1], axis=0),
                            element_offset=fh + nheads, oob_is_err=False)
                    Q = wp.tile([P, J * P], F32, tag="Q")
                    nc.vector.tensor_tensor(
                        out=Q[:].rearrange("p (j w) -> p j w", j=J),
                        in0=ap_nd(rl[:], 0, [(1, J), (0, P)]),
                        in1=ap_nd(iota_s[:], 0, [(0, J), (1, P)]),
                        op=mybir.AluOpType.is_equal)
                    lg = wp.tile([P, J * nheads], F32, tag="lg")
                    nc.vector.tensor_tensor(
                        out=lg[:], in0=ap_nd(pay[:], fh, [(FT, J), (1, nheads)]),
                        in1=edv[:], op=mybir.AluOpType.add)
                    nc.vector.scalar_tensor_tensor(
                        out=lg[:], in0=lg[:], scalar=0.2, in1=lg[:],
                        op0=mybir.AluOpType.mult, op1=mybir.AluOpType.max)
                    pv = wp.tile([P, J * nheads], F32, tag="pv")
                    nc.scalar.activation(out=pv[:], in_=lg[:],
                                         func=mybir.ActivationFunctionType.Exp)
                    rh = wp.tile([P, J * cols], F32, tag="rh")
                    nc.vector.tensor_tensor(
                        out=ap_nd(rh[:], 0, [(cols, J), (ch, nheads), (1, ch)]),
                        in0=ap_nd(pay[:], 0, [(FT, J), (ch, nheads), (1, ch)]),
                        in1=ap_nd(pv[:], 0, [(nheads, J), (1, nheads), (0, ch)]),
                        op=mybir.AluOpType.mult)
                    nc.vector.tensor_copy(
                        out=ap_nd(rh[:], fh, [(cols, J), (1, nheads)]),
                        in_=pv[:])
                    ps = pp.tile([P, cols], F32, tag="ps_edge")
                    for j in range(J):
                        nc.tensor.matmul(
                            out=ps[:], lhsT=Q[:, j * P:(j + 1) * P],
                            rhs=rh[:, j * cols:(j + 1) * cols],
                            start=(j == 0), stop=(j == J - 1))
                    dn = wp.tile([P, nheads], F32, tag="dn")
                    nc.vector.tensor_scalar_add(dn[:], ps[:, fh:fh + nheads],
                                                1e-16)
                    rc = wp.tile([P, nheads], F32, tag="rc")
                    nc.vector.reciprocal(rc[:], dn[:])
                    xr = wp.tile([P, fh], F32, tag="xr")
                    nc.vector.tensor_tensor(
                        out=ap_nd(xr[:], 0, [(ch, nheads), (1, ch)]),
                        in0=ap_nd(ps[:], 0, [(ch, nheads), (1, ch)]),
                        in1=ap_nd(rc[:], 0, [(1, nheads), (0, ch)]),
                        op=mybir.AluOpType.mult)
                    nc.vector.tensor_tensor(
                        out=xr[:], in0=xr[:], in1=b_s[:, :fh],
                        op=mybir.AluOpType.add)
                    nc.vector.tensor_scalar_max(xr[:], xr[:], 0.0)
                    pst = pp.tile([fh, P], F32, tag="pst")
                    nc.tensor.transpose(out=pst[:], in_=xr[:],
                                        identity=ident_s[:])
                    xrT = wp.tile([fh, P], F32, tag="xrT")
                    nc.scalar.copy(out=xrT[:], in_=pst[:])
                    if not final:
                        psn = pp.tile([P, FN], F32, tag="psn")
                        nc.tensor.matmul(out=psn[:], lhsT=xrT[:],
                                         rhs=Waug_next_s[:], start=True,
                                         stop=True)
                        hn = wp.tile([P, FN], F32, tag="hn")
                        nc.scalar.copy(out=hn[:], in_=psn[:])
                        nc.sync.dma_start(out=shard_next[w * P:(w + 1) * P, :],
                                          in_=hn[:])
                    else:
                        psn = pp.tile([P, NCLS], F32, tag="psn")
                        nc.tensor.matmul(out=psn[:], lhsT=xrT[:], rhs=linw_s[:],
                                         start=True, stop=True)
                        yo = wp.tile([P, NCLS], F32, tag="yo")
                        nc.vector.tensor_tensor(
                            out=yo[:], in0=psn[:], in1=linb_s[:],
                            op=mybir.AluOpType.add)
                        nc.sync.dma_start(out=out_ext[w * P:(w + 1) * P, :],
                                          in_=yo[:])

            edge_phase(table0, FA, H, shard1, FA, Waug1_s, b0_s, final=False)
            nc.sync.dma_start(out=shard1[npad:npad + 16, :], in_=dummy_s[:])
            allgather(shard1, table1)
            edge_phase(table1, FA, H, shard2, FA2, Waug2_s, b1_s, final=False)
            nc.sync.dma_start(out=shard2[npad:npad + 16, :], in_=dummy2_s[:])
            allgather(shard2, table2)
            edge_phase(table2, FA2, 1, None, None, None, b2_s, final=True)

    nc.compile()
    return nc


# ---------------------------------------------------------------- entry point

def kernel(**inputs):
    from concourse.bass_utils import run_bass_kernel_spmd
    global LAST_EXEC_NS
    N = N_NODES
    ncores = NCORES
    x = np.asarray(inputs['x'], np.float32)
    ei = np.asarray(inputs['edge_index'])
    loop = np.arange(N, dtype=np.int64)
    src = np.concatenate([np.asarray(ei[0], np.int64), loop])
    dst = np.concatenate([np.asarray(ei[1], np.int64), loop])
    meta, g_idx, ed_idx, rel_f = preprocess(src, dst, N, ncores)
    consts = make_const_inputs(inputs)
    nloc, npad = meta['nloc'], meta['npad']

    nc = build_program(meta, ncores)

    in_maps = []
    for c in range(ncores):
        xc = np.zeros((npad, 128), np.float32)
        xc[:nloc] = x[c * nloc:(c + 1) * nloc]
        m = dict(consts)
        m['xT'] = np.ascontiguousarray(xc.T)
        m['gidx'] = g_idx[c]
        m['edidx'] = ed_idx[c]
        m['rel'] = rel_f[c]
        in_maps.append(m)

    trace = _install_ntff_hook()
    res = run_bass_kernel_spmd(nc, in_maps, list(range(ncores)), trace=trace)
    LAST_EXEC_NS = res.exec_time_ns
    out = np.concatenate(
        [res.results[c]['out'][:nloc] for c in range(ncores)], axis=0)
    return np.ascontiguousarray(out.astype(np.float32))

